# revision 5
# baseline (speedup 1.0000x reference)
"""Bundle-adjustment projection kernel for Trainium2 (8 NeuronCores).

Strategy: edges are sorted by keyframe id (host-side index plumbing) and packed
into 512-slot single-kf blocks.  For each kf the host folds the camera
intrinsics into the pose once (A = [[FX*K0+CX*K2], [FY*K1+CY*K2], [K2]], a
per-parameter transform), so each edge's screen coords are u = A0.xh / A2.xh,
v = A1.xh / A2.xh.  32 blocks form a "superpack": one PE matmul with a
block-diagonal stationary lhsT [128,128] computes all numerators/denominators
for 32*512 edges in 512 cycles (fp32r fast path).  Superpack pairs share
[128,512] DVE tiles: one reciprocal + one multiply per 32K edges.  Host
inverse-permutes the padded block layout back to edge order.
"""
import sys
sys.path.insert(0, "/opt/trn_rl_repo")

import numpy as np

FX, FY, CX, CY = 320.0, 320.0, 320.0, 240.0
N_MP, N_KF, M = 200000, 2000, 4000000
N_CORES = 8
BLK = 512                      # slots per block (single kf, matmul free dim)
BPS = 32                       # blocks per superpack (32*4 = 128 contraction)
SPC = 2                        # superpacks per DVE/out tile pair

_CACHE = {}


def _build(n_sp, n_rep=1, use_f32r=True):
    import concourse.bacc as bacc
    import concourse.mybir as mybir
    import concourse.tile as tile

    f32 = mybir.dt.float32
    mmdt = mybir.dt.float32r if use_f32r else mybir.dt.float32
    Alu = mybir.AluOpType
    n_pair = n_sp // SPC

    nc = bacc.Bacc(None, target_bir_lowering=False)
    v4_h = nc.dram_tensor("V4", [128, n_sp * BLK], mmdt, kind="ExternalInput")
    w_h = nc.dram_tensor("W", [128, n_sp * 128], mmdt, kind="ExternalInput")
    out_h = nc.dram_tensor("out", [128, n_pair * BLK], f32, kind="ExternalOutput")

    with tile.TileContext(nc) as tc:
        with (
            tc.tile_pool(name="work", bufs=3) as work,
            tc.tile_pool(name="psum", bufs=3, space="PSUM") as psump,
        ):
            for _rep in range(n_rep):
                for t in range(n_pair):
                    wt = work.tile([128, SPC * 128], mmdt, tag="w")
                    nc.sync.dma_start(
                        wt[:], w_h[:, t * SPC * 128:(t + 1) * SPC * 128])
                    uv = work.tile([128, BLK], f32, tag="uv")
                    for h in range(SPC):
                        s = t * SPC + h
                        vt = work.tile([128, BLK], mmdt, tag="v")
                        nc.sync.dma_start(
                            vt[:], v4_h[:, s * BLK:(s + 1) * BLK])
                        ps = psump.tile([128, BLK], f32, tag="ps")
                        nc.tensor.matmul(
                            out=ps[:, :],
                            lhsT=wt[:, h * 128:(h + 1) * 128],
                            rhs=vt[:], start=True, stop=True)
                        rec = work.tile([64, BLK], f32, tag="rec")
                        nc.vector.reciprocal_approx_fast(rec[:], ps[0:64, :])
                        nc.vector.tensor_tensor(
                            uv[64 * h:64 * h + 64, :], ps[64:128, :], rec[:],
                            op=Alu.mult)
                    nc.sync.dma_start(
                        out_h[:, t * BLK:(t + 1) * BLK], uv[:])
    nc.finalize()
    return nc


def _prep_inputs(tMP, tKF, kf_ids, mp_ids, idxKF, idxMP):
    tMP = np.asarray(tMP, np.float32)
    tKF = np.asarray(tKF, np.float32)
    idsKF = np.searchsorted(np.asarray(idxKF), np.asarray(kf_ids))
    idsMP = np.searchsorted(np.asarray(idxMP), np.asarray(mp_ids))
    n_kf = len(idxKF)

    perm = np.argsort(idsKF, kind="stable")
    kf_s = idsKF[perm]
    mp_s = idsMP[perm]

    counts = np.bincount(kf_s, minlength=n_kf)
    starts = np.concatenate([[0], np.cumsum(counts)])[:-1]
    nblk = -(-counts // BLK)              # ceil; 0 for empty kfs
    b_tot = int(nblk.sum())
    bpc = BPS * SPC                       # blocks per (superpack pair) unit
    n_sp = max(SPC, -(-b_tot // (N_CORES * BPS)))
    n_sp += (-n_sp) % SPC
    b_pad = N_CORES * BPS * n_sp

    # per-block kf and within-kf rank
    block_kf = np.repeat(np.arange(n_kf), nblk)
    blk_base = np.repeat(np.cumsum(nblk) - nblk, nblk)
    block_rank = np.arange(b_tot) - blk_base

    # slot -> sorted-edge index (pad slots duplicate the kf's last edge;
    # fully-dummy pad blocks duplicate sorted edge 0)
    cols = np.arange(BLK)
    pos = block_rank[:, None] * BLK + cols[None, :]          # [b_tot, BLK]
    lim = (counts[block_kf] - 1)[:, None]
    src = np.empty((b_pad, BLK), np.int64)
    src[:b_tot] = starts[block_kf][:, None] + np.minimum(pos, lim)
    src[b_tot:] = 0

    bkf = np.empty(b_pad, np.int64)
    bkf[:b_tot] = block_kf
    bkf[b_tot:] = kf_s[0]

    # folded per-kf projection matrices A [n_kf, 3, 4]
    A = np.empty((n_kf, 3, 4), np.float32)
    A[:, 0] = FX * tKF[:, 0, :] + CX * tKF[:, 2, :]
    A[:, 1] = FY * tKF[:, 1, :] + CY * tKF[:, 2, :]
    A[:, 2] = tKF[:, 2, :]

    tMPh = np.concatenate([tMP, np.ones((tMP.shape[0], 1), np.float32)], 1)

    n_pair = n_sp // SPC
    b_core = BPS * n_sp
    bb = np.arange(BPS)
    jj = np.arange(4)
    ii = np.arange(2)

    in_maps = []
    for c in range(N_CORES):
        seg = slice(c * b_core, (c + 1) * b_core)
        # V4: [n_sp, BPS, BLK] mp slots -> xh -> rows 4b+j
        xh = tMPh[mp_s[src[seg]]].reshape(n_sp, BPS, BLK, 4)
        v4 = np.ascontiguousarray(
            xh.transpose(0, 1, 3, 2).reshape(n_sp, 128, BLK)
            .transpose(1, 0, 2).reshape(128, n_sp * BLK))
        # W: [n_sp, 128, 128]; cols 0:64 nums, 64:128 dens
        Ab = A[bkf[seg]].reshape(n_sp, BPS, 3, 4)
        W = np.zeros((n_sp, 128, 128), np.float32)
        r4 = (4 * bb[:, None, None] + jj[None, :, None])      # [BPS,4,2]
        c2 = (2 * bb[:, None, None] + ii[None, None, :])
        # dens at out partitions 0:64 (reciprocal_approx_fast needs a
        # base-0 input AP), nums at 64:128
        W[:, r4, c2] = Ab[:, bb[:, None, None], 2, jj[None, :, None]]
        W[:, r4, 64 + c2] = Ab[:, bb[:, None, None], ii[None, None, :],
                               jj[None, :, None]]
        Wf = np.ascontiguousarray(
            W.transpose(1, 0, 2).reshape(128, n_sp * 128))
        in_maps.append({"V4": v4, "W": Wf})

    meta = (perm, src, n_sp, b_core)
    return in_maps, meta


def _unshard(outs, meta):
    perm, src, n_sp, b_core = meta
    n_pair = n_sp // SPC
    b_pad = src.shape[0]
    # slot (global block g, col c): core = g // b_core, gb = g % b_core,
    # s = gb // BPS, b = gb % BPS, t = s // SPC, h = s % SPC
    g = np.arange(b_pad)
    core = g // b_core
    gb = g % b_core
    s = gb // BPS
    b = gb % BPS
    t = s // SPC
    h = s % SPC
    rowu = 64 * h + 2 * b                                   # [b_pad]
    colbase = t * BLK                                       # [b_pad]
    stacked = np.stack(outs)                                # [8,128,n_pair*BLK]
    cols = np.arange(BLK)
    cidx = colbase[:, None] + cols[None, :]
    u = stacked[core[:, None], rowu[:, None], cidx]
    v = stacked[core[:, None], (rowu + 1)[:, None], cidx]
    res = np.empty((M, 2), np.float32)
    orig = perm[src]
    res[orig.ravel(), 0] = u.ravel()
    res[orig.ravel(), 1] = v.ravel()
    return res


def kernel(tMP, tKF, kf_ids, mp_ids, idxKF, idxMP):
    from concourse.bass_utils import run_bass_kernel_spmd

    in_maps, meta = _prep_inputs(tMP, tKF, kf_ids, mp_ids, idxKF, idxMP)
    n_sp = meta[2]
    key = ("nc", n_sp)
    if key not in _CACHE:
        _CACHE[key] = _build(n_sp)
    nc = _CACHE[key]
    res = run_bass_kernel_spmd(nc, in_maps, core_ids=list(range(N_CORES)))
    outs = [res.results[i]["out"] for i in range(N_CORES)]
    return _unshard(outs, meta)
